# revision 5
# baseline (speedup 1.0000x reference)
"""BinTokenizer kernel for Trainium2 (8 NeuronCores, data-parallel).

reference math: tokens = searchsorted(thresholds, clip(x, eps, 1-eps), 'right') - 1
with thresholds = linspace(0, 1, 257) in float32 == exactly i/256.

Key reduction (exact, proven + numpy-exhausted over all 2^16 high-half
patterns): the token depends only on the TOP 16 BITS of each f32 input.
bf16-truncation (round-toward-zero) can never cross a bin boundary because
every boundary i/256 is exactly representable in bf16 within each binade
(boundaries in [2^e, 2^(e+1)) need <= 8+e+1 <= 8 significand bits for e<=-1),
and the truncation grid in that binade (2^(e-7)) is at least as fine as the
bin grid (2^-8).  So the host hands the device only the high halves
(u16 view of f32, stride 2), HALVING device load traffic: 32 MiB/core loads
+ 16 MiB/core u8 stores = 48 MiB/core, vs 80 MiB/core for the f32 kernel.

Device math per element (bf16 xb): v = xb*256 is exact in f32 (power-of-two
scale, <=8 significand bits); v - (0.5 - 2^-16) is exact (needs <=24 bits);
RNE-to-u8 of that equals floor(v) for all v in [0,256) on the bf16 grid, and
the u8 convert's saturation handles x<0 -> 0 and x>=1 -> 255 exactly like
the reference's clip.  trn2 f32->int converts are RNE with saturation on
DVE/ACT/DMA alike (HW-probed by the prior f32 kernel, 0 mismatches).

Engine plan per tile [128 x 8192]: load u16 on the SP HWDGE ring, one
tensor_scalar (mult, subtract) with the SBUF AP bitcast u16->bf16, u8 store
on the ACT HWDGE ring.  Compute alternates DVE/ACT so neither engine's
stream (DVE 1x worst case ~8.6us/tile) gates the ~5us/tile DMA stream.
Expected floor: 48 MiB through the per-NC HBM path (~358-434 GB/s) ~ 110-140us
+ ~20us fixed NEFF envelope (BSP entry + IRAM loads + exit ladder).

Fallback: generic affine path on full f32 (exact floor for any convert
rounding) if thresholds are not the i/256 grid — same code as the previous
f32 kernel.
"""

import os
import sys

sys.path.insert(0, "/opt/trn_rl_repo")

import numpy as np

N_CORES = 8
B, T, D = 64, 4096, 512
PER_CORE = (B // N_CORES) * T * D  # 16,777,216 elements per core
P = 128                            # SBUF partitions
M = 8192                           # fast-path tile free dim (16 KiB/partition u16)
M_GENERAL = 2048                   # general path holds 5 extra tmp tiles, so smaller

MAGIC = 0.5 - 2.0**-16

LAST_RESULT = None  # BassKernelResults of the most recent run (for test.py)

_program_cache = {}


def _build_fast():
    """u16(=bf16 high-half) loads -> tensor_scalar(x*256 - MAGIC) -> u8 stores."""
    import concourse.bacc as bacc
    import concourse.tile as tile
    from concourse import mybir

    rows = PER_CORE // M
    ntiles = rows // P

    nc = bacc.Bacc("TRN2")
    F32, U8, U16 = mybir.dt.float32, mybir.dt.uint8, mybir.dt.uint16
    BF16 = mybir.dt.bfloat16
    Alu = mybir.AluOpType
    Act = mybir.ActivationFunctionType
    x = nc.dram_tensor("x", [rows, M], U16, kind="ExternalInput")
    y = nc.dram_tensor("y", [rows, M], U8, kind="ExternalOutput")
    xt = x.rearrange("(n p) m -> n p m", p=P)
    yt = y.rearrange("(n p) m -> n p m", p=P)

    def compute(idx, t_out, t_in):
        src = t_in[:].bitcast(BF16)
        if idx % 2 == 0:
            # DVE: u8 <- RNE(bf16 * 256 - MAGIC), saturating
            nc.vector.tensor_scalar(
                t_out[:], src, 256.0, MAGIC, Alu.mult, Alu.subtract
            )
        else:
            # ACT: same affine via activation Copy(scale*x + bias)
            nc.scalar.activation(t_out[:], src, Act.Copy, bias=-MAGIC, scale=256.0)

    with tile.TileContext(nc) as tc:
        with tc.tile_pool(name="io_in", bufs=4) as in_pool, tc.tile_pool(
            name="io_out", bufs=4
        ) as out_pool:
            # head/tail taper: short chunks at both ends so the first compute
            # starts early and the final load->compute->store chain drains fast
            HEAD = [1024, 1024, 2048, 4096]
            TAIL = [4096, 2048, 1024, 1024]
            k = 0
            for i in range(ntiles):
                chunks = HEAD if i == 0 else TAIL if i == ntiles - 1 else [M]
                off = 0
                for sub_m in chunks:
                    cols = slice(off, off + sub_m)
                    off += sub_m
                    t_in = in_pool.tile([P, sub_m], U16, tag="in")
                    nc.sync.dma_start(t_in[:], xt[i][:, cols])
                    t_out = out_pool.tile([P, sub_m], U8, tag="out")
                    compute(k, t_out, t_in)
                    nc.scalar.dma_start(yt[i][:, cols], t_out[:])
                    k += 1

    nc.finalize()
    return nc


def _build_general(scale: float, t0: float):
    """Affine binning exact for any f32 and any convert rounding mode."""
    import concourse.bacc as bacc
    import concourse.tile as tile
    from concourse import mybir

    m = M_GENERAL
    rows = PER_CORE // m
    ntiles = rows // P

    nc = bacc.Bacc("TRN2")
    F32, I32, U8 = mybir.dt.float32, mybir.dt.int32, mybir.dt.uint8
    Alu = mybir.AluOpType
    x = nc.dram_tensor("x", [rows, m], F32, kind="ExternalInput")
    y = nc.dram_tensor("y", [rows, m], U8, kind="ExternalOutput")
    xt = x.rearrange("(n p) m -> n p m", p=P)
    yt = y.rearrange("(n p) m -> n p m", p=P)

    with tile.TileContext(nc) as tc:
        with tc.tile_pool(name="io_in", bufs=4) as in_pool, tc.tile_pool(
            name="io_out", bufs=3
        ) as out_pool, tc.tile_pool(name="tmp", bufs=2) as tmp_pool:
            for i in range(ntiles):
                t_in = in_pool.tile([P, m], F32, tag="in")
                nc.sync.dma_start(t_in[:], xt[i])
                t_out = out_pool.tile([P, m], U8, tag="out")
                # v = (x - t0) * scale ; y0 = cvt(v)
                t_v = tmp_pool.tile([P, m], F32, tag="v")
                if t0 == 0.0:
                    nc.vector.tensor_scalar(
                        t_v[:], t_in[:], float(scale), None, Alu.mult
                    )
                else:
                    nc.vector.tensor_scalar(
                        t_v[:], t_in[:], float(t0), float(scale),
                        Alu.subtract, Alu.mult,
                    )
                t_y0 = tmp_pool.tile([P, m], I32, tag="y0")
                nc.vector.tensor_scalar(t_y0[:], t_v[:], 1.0, None, Alu.mult)
                # y0 back to f32 on the (otherwise idle) ACT engine
                t_y0f = tmp_pool.tile([P, m], F32, tag="y0f")
                nc.scalar.activation(
                    t_y0f[:], t_y0[:], mybir.ActivationFunctionType.Copy
                )
                t_gt = tmp_pool.tile([P, m], I32, tag="gt")
                nc.vector.tensor_tensor(t_gt[:], t_y0f[:], t_v[:], Alu.is_gt)
                t_y1 = tmp_pool.tile([P, m], I32, tag="y1")
                nc.vector.tensor_tensor(t_y1[:], t_y0[:], t_gt[:], Alu.subtract)
                nc.vector.tensor_scalar(
                    t_out[:], t_y1[:], 255, 0, Alu.min, Alu.max
                )  # clamp keeps the u8 convert in-range for any input
                nc.scalar.dma_start(yt[i], t_out[:])

    nc.finalize()
    return nc


def _uniform_grid(t: np.ndarray) -> bool:
    """thresholds exactly the i/256 grid on [0, 1]?"""
    return t.shape == (257,) and np.array_equal(
        t.astype(np.float64), np.arange(257) / 256.0
    )


def _run_prestaged(nc, global_ins: dict, n_cores: int) -> list[dict]:
    """Execute via PJRT with every device buffer resident BEFORE launch.

    bass_utils.run_bass_kernel_spmd -> bass2jax.run_bass_via_pjrt hands jit
    plain numpy arrays, so the 8 per-core H2D uploads (input shards + the
    donated zero output buffers) are issued as part of the same dispatch and
    can still be in flight on some HBM stacks while other cores are already
    executing -- measured as +20-26us of DMA slowdown on 1-2 straggler cores
    (idle gaps on the load ring + stretched packets).  Here we device_put the
    sharded inputs and create the donated zero outputs on-device, block until
    everything is resident, and only then launch the NEFF.
    """
    import jax
    import jax.numpy as jnp
    from jax.experimental.shard_map import shard_map
    from jax.sharding import Mesh, NamedSharding, PartitionSpec

    from concourse import bass2jax, mybir

    bass2jax.install_neuronx_cc_hook()
    assert nc.partition_id_tensor is None and nc.dbg_addr is None

    in_names: list[str] = []
    out_names: list[str] = []
    out_avals: list = []
    for alloc in nc.m.functions[0].allocations:
        if not isinstance(alloc, mybir.MemoryLocationSet):
            continue
        name = alloc.memorylocations[0].name
        if alloc.kind == "ExternalInput":
            in_names.append(name)
        elif alloc.kind == "ExternalOutput":
            out_names.append(name)
            out_avals.append(
                jax.core.ShapedArray(tuple(alloc.tensor_shape), mybir.dt.np(alloc.dtype))
            )
    n_params, n_outs = len(in_names), len(out_avals)
    bind_in_names = tuple(in_names + out_names)

    def _body(*args):
        outs = bass2jax._bass_exec_p.bind(
            *args,
            out_avals=tuple(out_avals),
            in_names=bind_in_names,
            out_names=tuple(out_names),
            lowering_input_output_aliases=(),
            sim_require_finite=True,
            sim_require_nnan=True,
            nc=nc,
        )
        return tuple(outs)

    devices = jax.devices()[:n_cores]
    assert len(devices) == n_cores
    mesh = Mesh(np.asarray(devices), ("core",))
    sh = NamedSharding(mesh, PartitionSpec("core"))
    fn = jax.jit(
        shard_map(
            _body,
            mesh=mesh,
            in_specs=(PartitionSpec("core"),) * (n_params + n_outs),
            out_specs=(PartitionSpec("core"),) * n_outs,
            check_rep=False,
        ),
        donate_argnums=tuple(range(n_params, n_params + n_outs)),
        keep_unused=True,
    )

    staged = [jax.device_put(global_ins[name], sh) for name in in_names]

    def _make_zeros():
        return tuple(
            jnp.zeros((n_cores * a.shape[0], *a.shape[1:]), a.dtype) for a in out_avals
        )

    zeros = list(jax.jit(_make_zeros, out_shardings=(sh,) * n_outs)())
    jax.block_until_ready(staged + zeros)

    out_arrs = fn(*staged, *zeros)
    jax.block_until_ready(out_arrs)
    return [
        {
            name: np.asarray(out_arrs[i]).reshape(n_cores, *out_avals[i].shape)[c]
            for i, name in enumerate(out_names)
        }
        for c in range(n_cores)
    ]


def _execute(nc, global_ins: dict, in_maps: list[dict], n_cores: int):
    """Prestaged exec with NTFF tracing glue matching run_bass_kernel_spmd's
    axon branch; falls back to stock run_bass_kernel_spmd on any failure."""
    import glob
    import tempfile

    from concourse import bass_utils as BU

    try:
        if not BU.axon_active():
            raise RuntimeError("native path: use stock runner")

        trace = BU.checkenv("BASS_TRACE") and not BU.checkenv("BASS_NEVER_TRACE")
        hook = None
        if trace:
            try:
                from antenv.axon_hooks import get_axon_ntff_profile_hook

                hook = get_axon_ntff_profile_hook()
            except Exception:
                hook = None
        if hook is None:
            results = _run_prestaged(nc, global_ins, n_cores)
            return BU.BassKernelResults(
                results=results,
                instructions_and_trace=None,
                profile_json=None,
                exec_time_ns=None,
            )

        neff_dir = tempfile.mkdtemp()
        core_ids = list(range(n_cores))
        trace_model_indices = (
            core_ids if BU.env_bass_perfetto_profile_all_cores() else [0]
        )
        with hook(neff_dir, trace_model_indices):
            results = _run_prestaged(nc, global_ins, n_cores)

        ntffs = glob.glob(os.path.join(neff_dir, "*_body*.ntff"))
        if not ntffs:
            return BU.BassKernelResults(
                results=results,
                instructions_and_trace=None,
                profile_json=None,
                exec_time_ns=None,
            )
        sharepath = BU.upload_artifacts(neff_dir)
        profile = BU.gauge.profiler.Profile(
            profile_path=BU.FishPath(neff_dir),
            kernel_dev_mode=True,
            profile_on_exit=False,
            bass_kernel=nc.m,
            offline_processing=True,
            fname="*_body*",
            metadata={"artifacts_path": sharepath},
        )
        return BU._process_ntff_profile(
            profile, neff_dir, nc, core_ids, None, False, {}, trace_events=False
        ).as_bass_kernel_results(results)
    except Exception:
        from concourse.bass_utils import run_bass_kernel_spmd

        return run_bass_kernel_spmd(nc, in_maps, list(range(n_cores)))


def kernel(inputs: np.ndarray, thresholds: np.ndarray) -> np.ndarray:
    global LAST_RESULT

    x = np.asarray(inputs, dtype=np.float32)
    t = np.asarray(thresholds, dtype=np.float32)

    if _uniform_grid(t):
        # high 16 bits of each f32 (little-endian: odd u16 halves) — exact
        hi = x.reshape(-1).view(np.uint16)[1::2]
        flat = np.ascontiguousarray(hi)
        key = ("fast",)
        if key not in _program_cache:
            _program_cache[key] = _build_fast()
        nc = _program_cache[key]
        rows, m = PER_CORE // M, M
    else:
        if not x.flags.c_contiguous:
            x = np.ascontiguousarray(x)
        td = t.astype(np.float64)
        scale = float(1.0 / (td[1] - td[0]))
        t0 = float(td[0])
        key = ("general", scale, t0)
        if key not in _program_cache:
            _program_cache[key] = _build_general(scale, t0)
        nc = _program_cache[key]
        rows, m = PER_CORE // M_GENERAL, M_GENERAL
        flat = x

    shards = flat.reshape(N_CORES, rows, m)
    global_ins = {"x": flat.reshape(N_CORES * rows, m)}
    in_maps = [{"x": shards[c]} for c in range(N_CORES)]
    res = _execute(nc, global_ins, in_maps, N_CORES)
    LAST_RESULT = res

    out = np.empty((N_CORES, rows, m), dtype=np.int32)
    for c in range(N_CORES):
        out[c] = res.results[c]["y"]
    return out.reshape(B, T, D)


# revision 10
# speedup vs baseline: 1.0299x; 1.0299x over previous
"""BinTokenizer kernel for Trainium2 (8 NeuronCores, data-parallel).

reference math: tokens = searchsorted(thresholds, clip(x, eps, 1-eps), 'right') - 1
with thresholds = linspace(0, 1, 257) in float32 == exactly i/256.

Key reduction (exact, proven + numpy-exhausted over all 2^16 high-half
patterns): the token depends only on the TOP 16 BITS of each f32 input.
bf16-truncation (round-toward-zero) can never cross a bin boundary because
every boundary i/256 is exactly representable in bf16 within each binade
(boundaries in [2^e, 2^(e+1)) need <= 8+e+1 <= 8 significand bits for e<=-1),
and the truncation grid in that binade (2^(e-7)) is at least as fine as the
bin grid (2^-8).  So the host hands the device only the high halves
(u16 view of f32, stride 2), HALVING device load traffic: 32 MiB/core loads
+ 16 MiB/core u8 stores = 48 MiB/core, vs 80 MiB/core for the f32 kernel.

Device math per element (bf16 xb): v = xb*256 is exact in f32 (power-of-two
scale, <=8 significand bits); v - (0.5 - 2^-16) is exact (needs <=24 bits);
RNE-to-u8 of that equals floor(v) for all v in [0,256) on the bf16 grid, and
the u8 convert's saturation handles x<0 -> 0 and x>=1 -> 255 exactly like
the reference's clip.  trn2 f32->int converts are RNE with saturation on
DVE/ACT/DMA alike (HW-probed by the prior f32 kernel, 0 mismatches).

Engine plan per tile [128 x 8192]: load u16 on the SP HWDGE ring, one
tensor_scalar (mult, subtract) with the SBUF AP bitcast u16->bf16, u8 store
on the ACT HWDGE ring.  Compute alternates DVE/ACT so neither engine's
stream (DVE 1x worst case ~8.6us/tile) gates the ~5us/tile DMA stream.
Expected floor: 48 MiB through the per-NC HBM path (~358-434 GB/s) ~ 110-140us
+ ~20us fixed NEFF envelope (BSP entry + IRAM loads + exit ladder).

Fallback: generic affine path on full f32 (exact floor for any convert
rounding) if thresholds are not the i/256 grid — same code as the previous
f32 kernel.
"""

import os
import sys

sys.path.insert(0, "/opt/trn_rl_repo")

import numpy as np

N_CORES = 8
B, T, D = 64, 4096, 512
PER_CORE = (B // N_CORES) * T * D  # 16,777,216 elements per core
P = 128                            # SBUF partitions
M = 8192                           # fast-path tile free dim (16 KiB/partition u16)
M_GENERAL = 2048                   # general path holds 5 extra tmp tiles, so smaller

MAGIC = 0.5 - 2.0**-16

LAST_RESULT = None  # BassKernelResults of the most recent run (for test.py)
LAST_PATH = None    # "prestaged" | "fallback" (diagnostics)

_program_cache = {}


def _build_fast():
    """u16(=bf16 high-half) loads -> tensor_scalar(x*256 - MAGIC) -> u8 stores."""
    import concourse.bacc as bacc
    import concourse.tile as tile
    from concourse import mybir

    rows = PER_CORE // M
    ntiles = rows // P

    nc = bacc.Bacc("TRN2")
    F32, U8, U16 = mybir.dt.float32, mybir.dt.uint8, mybir.dt.uint16
    BF16 = mybir.dt.bfloat16
    Alu = mybir.AluOpType
    Act = mybir.ActivationFunctionType
    x = nc.dram_tensor("x", [rows, M], U16, kind="ExternalInput")
    y = nc.dram_tensor("y", [rows, M], U8, kind="ExternalOutput")
    xt = x.rearrange("(n p) m -> n p m", p=P)
    yt = y.rearrange("(n p) m -> n p m", p=P)

    def compute(idx, t_out, t_in):
        src = t_in[:].bitcast(BF16)
        if idx % 2 == 0:
            # DVE: u8 <- RNE(bf16 * 256 - MAGIC), saturating
            nc.vector.tensor_scalar(
                t_out[:], src, 256.0, MAGIC, Alu.mult, Alu.subtract
            )
        else:
            # ACT: same affine via activation Copy(scale*x + bias)
            nc.scalar.activation(t_out[:], src, Act.Copy, bias=-MAGIC, scale=256.0)

    with tile.TileContext(nc) as tc:
        with tc.tile_pool(name="io_in", bufs=4) as in_pool, tc.tile_pool(
            name="io_out", bufs=4
        ) as out_pool:
            # head/tail taper: short chunks at both ends so the first compute
            # starts early and the final load->compute->store chain drains fast
            HEAD = [1024, 1024, 2048, 4096]
            TAIL = [4096, 2048, 1024, 1024]
            k = 0
            for i in range(ntiles):
                chunks = HEAD if i == 0 else TAIL if i == ntiles - 1 else [M]
                off = 0
                for sub_m in chunks:
                    cols = slice(off, off + sub_m)
                    off += sub_m
                    t_in = in_pool.tile([P, sub_m], U16, tag="in")
                    nc.sync.dma_start(t_in[:], xt[i][:, cols])
                    t_out = out_pool.tile([P, sub_m], U8, tag="out")
                    compute(k, t_out, t_in)
                    nc.scalar.dma_start(yt[i][:, cols], t_out[:])
                    k += 1

    nc.finalize()
    return nc


def _build_general(scale: float, t0: float):
    """Affine binning exact for any f32 and any convert rounding mode."""
    import concourse.bacc as bacc
    import concourse.tile as tile
    from concourse import mybir

    m = M_GENERAL
    rows = PER_CORE // m
    ntiles = rows // P

    nc = bacc.Bacc("TRN2")
    F32, I32, U8 = mybir.dt.float32, mybir.dt.int32, mybir.dt.uint8
    Alu = mybir.AluOpType
    x = nc.dram_tensor("x", [rows, m], F32, kind="ExternalInput")
    y = nc.dram_tensor("y", [rows, m], U8, kind="ExternalOutput")
    xt = x.rearrange("(n p) m -> n p m", p=P)
    yt = y.rearrange("(n p) m -> n p m", p=P)

    with tile.TileContext(nc) as tc:
        with tc.tile_pool(name="io_in", bufs=4) as in_pool, tc.tile_pool(
            name="io_out", bufs=3
        ) as out_pool, tc.tile_pool(name="tmp", bufs=2) as tmp_pool:
            for i in range(ntiles):
                t_in = in_pool.tile([P, m], F32, tag="in")
                nc.sync.dma_start(t_in[:], xt[i])
                t_out = out_pool.tile([P, m], U8, tag="out")
                # v = (x - t0) * scale ; y0 = cvt(v)
                t_v = tmp_pool.tile([P, m], F32, tag="v")
                if t0 == 0.0:
                    nc.vector.tensor_scalar(
                        t_v[:], t_in[:], float(scale), None, Alu.mult
                    )
                else:
                    nc.vector.tensor_scalar(
                        t_v[:], t_in[:], float(t0), float(scale),
                        Alu.subtract, Alu.mult,
                    )
                t_y0 = tmp_pool.tile([P, m], I32, tag="y0")
                nc.vector.tensor_scalar(t_y0[:], t_v[:], 1.0, None, Alu.mult)
                # y0 back to f32 on the (otherwise idle) ACT engine
                t_y0f = tmp_pool.tile([P, m], F32, tag="y0f")
                nc.scalar.activation(
                    t_y0f[:], t_y0[:], mybir.ActivationFunctionType.Copy
                )
                t_gt = tmp_pool.tile([P, m], I32, tag="gt")
                nc.vector.tensor_tensor(t_gt[:], t_y0f[:], t_v[:], Alu.is_gt)
                t_y1 = tmp_pool.tile([P, m], I32, tag="y1")
                nc.vector.tensor_tensor(t_y1[:], t_y0[:], t_gt[:], Alu.subtract)
                nc.vector.tensor_scalar(
                    t_out[:], t_y1[:], 255, 0, Alu.min, Alu.max
                )  # clamp keeps the u8 convert in-range for any input
                nc.scalar.dma_start(yt[i], t_out[:])

    nc.finalize()
    return nc


def _uniform_grid(t: np.ndarray) -> bool:
    """thresholds exactly the i/256 grid on [0, 1]?"""
    return t.shape == (257,) and np.array_equal(
        t.astype(np.float64), np.arange(257) / 256.0
    )


def _run_prestaged(nc, global_ins: dict, n_cores: int) -> list[dict]:
    """Execute via PJRT with every device buffer resident BEFORE launch.

    bass_utils.run_bass_kernel_spmd -> bass2jax.run_bass_via_pjrt hands jit
    plain numpy arrays, so the 8 per-core H2D uploads (input shards + the
    donated zero output buffers) are issued as part of the same dispatch and
    can still be in flight on some HBM stacks while other cores are already
    executing -- measured as +20-26us of DMA slowdown on 1-2 straggler cores
    (idle gaps on the load ring + stretched packets).  Here we device_put the
    sharded inputs and create the donated zero outputs on-device, block until
    everything is resident, and only then launch the NEFF.
    """
    import jax
    import jax.numpy as jnp
    from jax.experimental.shard_map import shard_map
    from jax.sharding import Mesh, NamedSharding, PartitionSpec

    from concourse import bass2jax, mybir

    bass2jax.install_neuronx_cc_hook()
    assert nc.partition_id_tensor is None and nc.dbg_addr is None

    in_names: list[str] = []
    out_names: list[str] = []
    out_avals: list = []
    for alloc in nc.m.functions[0].allocations:
        if not isinstance(alloc, mybir.MemoryLocationSet):
            continue
        name = alloc.memorylocations[0].name
        if alloc.kind == "ExternalInput":
            in_names.append(name)
        elif alloc.kind == "ExternalOutput":
            out_names.append(name)
            out_avals.append(
                jax.core.ShapedArray(tuple(alloc.tensor_shape), mybir.dt.np(alloc.dtype))
            )
    n_params, n_outs = len(in_names), len(out_avals)
    bind_in_names = tuple(in_names + out_names)

    def _body(*args):
        outs = bass2jax._bass_exec_p.bind(
            *args,
            out_avals=tuple(out_avals),
            in_names=bind_in_names,
            out_names=tuple(out_names),
            lowering_input_output_aliases=(),
            sim_require_finite=True,
            sim_require_nnan=True,
            nc=nc,
        )
        return tuple(outs)

    devices = jax.devices()[:n_cores]
    assert len(devices) == n_cores
    mesh = Mesh(np.asarray(devices), ("core",))
    sh = NamedSharding(mesh, PartitionSpec("core"))
    fn = jax.jit(
        shard_map(
            _body,
            mesh=mesh,
            in_specs=(PartitionSpec("core"),) * (n_params + n_outs),
            out_specs=(PartitionSpec("core"),) * n_outs,
            check_rep=False,
        ),
        donate_argnums=tuple(range(n_params, n_params + n_outs)),
        keep_unused=True,
    )

    staged = [jax.device_put(global_ins[name], sh) for name in in_names]

    def _make_zeros():
        return tuple(
            jnp.zeros((n_cores * a.shape[0], *a.shape[1:]), a.dtype) for a in out_avals
        )

    zeromaker = jax.jit(_make_zeros, out_shardings=(sh,) * n_outs)

    def launch():
        zeros = list(zeromaker())
        jax.block_until_ready(staged + zeros)
        out_arrs = fn(*staged, *zeros)
        jax.block_until_ready(out_arrs)
        return out_arrs

    def gather(out_arrs):
        return [
            {
                name: np.asarray(out_arrs[i]).reshape(n_cores, *out_avals[i].shape)[c]
                for i, name in enumerate(out_names)
            }
            for c in range(n_cores)
        ]

    return launch, gather


def _execute(nc, global_ins: dict, in_maps: list[dict], n_cores: int):
    """Prestaged exec with NTFF tracing glue matching run_bass_kernel_spmd's
    axon branch; falls back to stock run_bass_kernel_spmd on any failure."""
    import glob
    import tempfile

    from concourse import bass_utils as BU

    global LAST_PATH
    LAST_PATH = "prestaged"
    try:
        if not BU.axon_active():
            raise RuntimeError("native path: use stock runner")

        trace = BU.checkenv("BASS_TRACE") and not BU.checkenv("BASS_NEVER_TRACE")
        hook = None
        if trace:
            try:
                from antenv.axon_hooks import get_axon_ntff_profile_hook

                hook = get_axon_ntff_profile_hook()
            except Exception:
                hook = None
        if hook is None:
            results = _run_prestaged(nc, global_ins, n_cores)
            return BU.BassKernelResults(
                results=results,
                instructions_and_trace=None,
                profile_json=None,
                exec_time_ns=None,
            )

        neff_dir = tempfile.mkdtemp()
        core_ids = list(range(n_cores))
        trace_model_indices = (
            core_ids if BU.env_bass_perfetto_profile_all_cores() else [0]
        )
        with hook(neff_dir, trace_model_indices):
            results = _run_prestaged(nc, global_ins, n_cores)

        ntffs = glob.glob(os.path.join(neff_dir, "*_body*.ntff"))
        if not ntffs:
            return BU.BassKernelResults(
                results=results,
                instructions_and_trace=None,
                profile_json=None,
                exec_time_ns=None,
            )
        sharepath = BU.upload_artifacts(neff_dir)
        profile = BU.gauge.profiler.Profile(
            profile_path=BU.FishPath(neff_dir),
            kernel_dev_mode=True,
            profile_on_exit=False,
            bass_kernel=nc.m,
            offline_processing=True,
            fname="*_body*",
            metadata={"artifacts_path": sharepath},
        )
        return BU._process_ntff_profile(
            profile, neff_dir, nc, core_ids, None, False, {}, trace_events=False
        ).as_bass_kernel_results(results)
    except Exception:
        LAST_PATH = "fallback"
        from concourse.bass_utils import run_bass_kernel_spmd

        return run_bass_kernel_spmd(nc, in_maps, list(range(n_cores)))


def kernel(inputs: np.ndarray, thresholds: np.ndarray) -> np.ndarray:
    global LAST_RESULT

    x = np.asarray(inputs, dtype=np.float32)
    t = np.asarray(thresholds, dtype=np.float32)

    if _uniform_grid(t):
        # high 16 bits of each f32 (little-endian: odd u16 halves) — exact
        hi = x.reshape(-1).view(np.uint16)[1::2]
        flat = np.ascontiguousarray(hi)
        key = ("fast",)
        if key not in _program_cache:
            _program_cache[key] = _build_fast()
        nc = _program_cache[key]
        rows, m = PER_CORE // M, M
    else:
        if not x.flags.c_contiguous:
            x = np.ascontiguousarray(x)
        td = t.astype(np.float64)
        scale = float(1.0 / (td[1] - td[0]))
        t0 = float(td[0])
        key = ("general", scale, t0)
        if key not in _program_cache:
            _program_cache[key] = _build_general(scale, t0)
        nc = _program_cache[key]
        rows, m = PER_CORE // M_GENERAL, M_GENERAL
        flat = x

    shards = flat.reshape(N_CORES, rows, m)
    global_ins = {"x": flat.reshape(N_CORES * rows, m)}
    in_maps = [{"x": shards[c]} for c in range(N_CORES)]
    res = _execute(nc, global_ins, in_maps, N_CORES)
    LAST_RESULT = res

    out = np.empty((N_CORES, rows, m), dtype=np.int32)
    for c in range(N_CORES):
        out[c] = res.results[c]["y"]
    return out.reshape(B, T, D)
